# revision 35
# baseline (speedup 1.0000x reference)
"""Trainium2 Bass kernel for ConvPolicy14 (dense_cnn, 93 -> 40 policy net).

The network is tiny (~4.6K MACs): every conv/pool/upsample/concat is folded
(host-side, input-independently) into a chain of 8 dense affine layers run as
TensorE matmuls with PSUM accumulation and ScalarE tanh between them:

    h1 = tanh(M1 v0 + b1)            v0 = jcat flattened (84)
    h2 = tanh(M2 h1 + b2)
    h3 = tanh(M3 h2 + b3)            M3 = conv3_toeplitz @ avgpool
    h4 = tanh(M4 h3 + b4)
    h5 = tanh(M5 (h4 + ext) + b5)    ext = [psi, x47, x52]
    h6 = tanh(M6a h5 + M6b h3 + b6)
    h7 = tanh(M7a h6 + M7b h2 + b7)  M7a folds nearest-upsample
    out = M8a h7 + M8b v0 + b8       (40,) final, no tanh

This version is raw Bass (no Tile framework): one basic block, manual
semaphores, engine streams ordered by hand, everything bf16 except PSUM and
the final output (rel tol is 2e-2; measured ~2.4e-3).

- Biases ride inside the main matmuls: Ht is zeroed once and row 32 set to
  1.0 (SBUF partition bases must be quadrant-aligned), so each layer's lhsT
  gets bias in contraction row 32 (L1/L8 use the v0[0]:=1.0 slot). No
  separate bias matmuls.
- psi = atan2(qz,qw) + atan2(qx,qy) is computed as
  arctan(q0)+arctan(q1)+pi*c0+pi*c1 with c_f = sign(num_f)*[den_f<0]:
  7 tiny DVE ops put [q0, q1, c0, c1] in partitions 0:4 (tensor_tensor ops
  only - the dual-immediate tensor_scalar form mis-executes on [n,1]
  layouts - and sem-chained, since back-to-back same-engine RAW/WAW through
  SBUF is not interlocked), one 4-element ACT Arctan maps them
  (arctan(+-1) = +-pi/4 exactly; the correction rows of the L5 lhsT carry
  4x the psi weight row), and the L5 matmul sums the partitions.
- The last layer swaps matmul operand roles (stationary = h7/v0 column,
  moving = Kx40 weight block) so the result lands as a [1,40] PSUM row:
  the output DMA moves ONE contiguous 160B descriptor instead of 40
  scattered 4B partition writes, cutting its completion wait by ~3us.
- The Bass-init const memsets + all-engine barrier are dropped from the IR
  (activation biases are APs into a zeroed f32 column, so nothing
  references the const tensors); engines go straight from the runtime
  preamble into kernel work.
- Input and output DMAs trigger from the Activation engine (earliest
  preamble among HWDGE engines / idle after tanh7), with the single ACT
  table load (one set covers Tanh+Arctan) hidden under the input flight.
  The final PSUM->SBUF copy runs on DVE to keep the ACT queue short.
  Skipping the output-DMA completion wait is NOT possible: the NEFF must
  not exit with an unwaited in-flight DMA (NRT_EXEC_UNIT_UNRECOVERABLE).

Per the sharding hint the problem is too small to shard: all 8 cores run the
same program; core 0's output is returned.

Measured: 24783ns (prior Tile-based kernel) -> 15782ns, rel err 2.4e-3.
"""

import numpy as np

F32 = np.float32

# ---------------------------------------------------------------------------
# Block layout (single DMA'd constant/input block, 128 partitions x _B_COLS)
# lhsT layout: K contraction rows on partitions, M output cols in free dim.
# ---------------------------------------------------------------------------
_C_L1 = 0        # (84, 28)  row0 = b1 (v0[0] := 1.0 trick), row1 = 0
_C_L2 = 28       # (33, 28)  row32 = b2
_C_L3 = 56       # (33, 12)  row32 = b3
_C_L4 = 68       # (33, 3)   rows 12:32 = 0, row32 = b4
_C_L5E = 71      # (3, 12)   rows = [M5.T[x47], M5.T[x52], b5]
_C_L5P = 83      # (4, 12)   rows = [w, w, 4w, 4w], w = M5.T[psi]
_C_L5A = 95      # (3, 12)   M5.T (h4 side)
_C_L6H3 = 107    # (12, 12)  M6[:, 12:].T
_C_L6H5 = 119    # (33, 12)  rows 0:12 = M6[:, :12].T, row32 = b6
_C_L7H2 = 131    # (33, 28)  rows 0:28 = M7b.T, row32 = b7
_C_L7H6 = 159    # (12, 28)  M7a.T
_C_L8B = 187     # (84, 40)  row0 = b8, row1 = 0
_C_L8A = 227     # (28, 40)  M8[:, :28].T
_C_V0 = 267      # v0 col (p0 = 1.0, p2:42 = x[7:47], p44:84 = x[53:93])
_C_EXT = 268     # ext col [x47, x52, 1.0] at p0:3
_C_DEN = 269     # [qw, qy, qw, qy] at p0:4
_C_NUM = 270     # [qz, qx, qz, qx] at p0:4
_C_Z = 271       # all-zero col (activation bias AP)
_B_COLS = 272
_B_ROWS = 85     # partitions actually read (max K = 84, bias row 32)

# Ht columns: h1..h7 at 0..6, psi parts at 7; row 32 of cols 0:7 = 1.0
_PSIC = 7
_BR = 32         # bias row (quadrant-aligned partition base)


def _toeplitz_conv(cw, L):
    """Conv1d pad=1 k=3: out[(o,l)] = sum_{c,k} cw[o,c,k] x[(c,l+k-1)]."""
    O, C, _ = cw.shape
    M = np.zeros((O * L, C * L), F32)
    for o in range(O):
        for l in range(L):
            for c in range(C):
                for k in range(3):
                    m = l + k - 1
                    if 0 <= m < L:
                        M[o * L + l, c * L + m] = cw[o, c, k]
    return M


def _toeplitz_deconv(dw, L):
    """ConvTranspose1d pad=1 k=3 s=1: out[(o,l)] = sum dw[c,o,1-m+l] x[(c,m)]."""
    C, O, _ = dw.shape
    M = np.zeros((O * L, C * L), F32)
    for o in range(O):
        for l in range(L):
            for c in range(C):
                for m in range(L):
                    k = 1 - m + l
                    if 0 <= k < 3:
                        M[o * L + l, c * L + m] = dw[c, o, k]
    return M


def _build_w_block(w):
    c1w, c1b = w["conv1_w"], w["conv1_b"]
    c2w, c2b = w["conv2_w"], w["conv2_b"]
    c3w, c3b = w["conv3_w"], w["conv3_b"]
    c4w, c4b = w["conv4_w"], w["conv4_b"]
    d1w, d1b = w["deconv1_w"], w["deconv1_b"]
    d2w, d2b = w["deconv2_w"], w["deconv2_b"]
    d3w, d3b = w["deconv3_w"], w["deconv3_b"]
    d4w, d4b = w["deconv4_w"], w["deconv4_b"]

    M1 = _toeplitz_conv(c1w, 7)                     # (28, 84)
    M2 = _toeplitz_conv(c2w, 7)                     # (28, 28)

    # adaptive avg pool (4,7)->(4,3), windows [0:3],[2:5],[4:7]
    P = np.zeros((12, 28), F32)
    for c in range(4):
        for j in range(3):
            P[c * 3 + j, c * 7 + 2 * j: c * 7 + 2 * j + 3] = 1.0 / 3.0
    T3 = np.zeros((12, 12), F32)
    for o in range(4):
        for j in range(3):
            for c in range(4):
                for k in range(3):
                    jp = j + k - 1
                    if 0 <= jp < 3:
                        T3[o * 3 + j, c * 3 + jp] = c3w[o, c, k]
    M3 = (T3.astype(np.float64) @ P.astype(np.float64)).astype(F32)  # (12, 28)

    M4 = np.zeros((3, 12), F32)                     # conv4 pad0 L3->1
    for o in range(3):
        for c in range(4):
            M4[o, c * 3: c * 3 + 3] = c4w[o, c, :]

    M5 = np.zeros((12, 3), F32)                     # deconv1 L1->3
    for o in range(4):
        for l in range(3):
            for c in range(3):
                M5[o * 3 + l, c] = d1w[c, o, l]

    M6 = _toeplitz_deconv(d2w, 3)                   # (12, 24): [h5 | h3]

    T7 = _toeplitz_deconv(d3w, 7)                   # (28, 56)
    g = [0, 0, 0, 1, 1, 2, 2]                       # nearest-upsample 3->7
    U = np.zeros((28, 12), F32)
    for c in range(4):
        for l in range(7):
            U[c * 7 + l, c * 3 + g[l]] = 1.0
    M7a = (T7[:, :28].astype(np.float64) @ U.astype(np.float64)).astype(F32)
    M7b = np.ascontiguousarray(T7[:, 28:])          # (28, 28)

    M8 = _toeplitz_deconv(d4w, 7)[2:, :]            # (40, 112): acts[2:]
    b8 = np.repeat(d4b, 7).astype(F32)[2:]

    b1 = np.repeat(c1b, 7).astype(F32)
    b2 = np.repeat(c2b, 7).astype(F32)
    b3 = np.repeat(c3b, 3).astype(F32)
    b5 = np.repeat(d1b, 3).astype(F32)
    b6 = np.repeat(d2b, 3).astype(F32)
    b7 = np.repeat(d3b, 7).astype(F32)

    blk = np.zeros((128, _B_COLS), F32)

    def put(col, mat):
        K, M = mat.shape
        blk[:K, col:col + M] = mat

    lhsT1 = M1.T.copy()                 # (84, 28)
    lhsT1[0, :] = b1                    # v0[0]/v0[1] structural zeros
    lhsT1[1, :] = 0.0
    put(_C_L1, lhsT1)

    put(_C_L2, M2.T)
    blk[_BR, _C_L2:_C_L2 + 28] = b2

    put(_C_L3, M3.T)
    blk[_BR, _C_L3:_C_L3 + 12] = b3

    put(_C_L4, M4.T)
    blk[_BR, _C_L4:_C_L4 + 3] = np.asarray(c4b, F32)

    put(_C_L5E, np.stack([M5[:, 1], M5[:, 2], b5]))          # (3, 12)
    put(_C_L5P, np.stack([M5[:, 0], M5[:, 0],
                          4 * M5[:, 0], 4 * M5[:, 0]]))      # (4, 12)
    put(_C_L5A, M5.T)

    put(_C_L6H3, M6[:, 12:].T)
    put(_C_L6H5, M6[:, :12].T)
    blk[_BR, _C_L6H5:_C_L6H5 + 12] = b6

    put(_C_L7H2, M7b.T)
    blk[_BR, _C_L7H2:_C_L7H2 + 28] = b7
    put(_C_L7H6, M7a.T)

    lhsT8b = M8[:, 28:].T.copy()        # (84, 40)
    lhsT8b[0, :] = b8
    lhsT8b[1, :] = 0.0
    put(_C_L8B, lhsT8b)
    put(_C_L8A, M8[:, :28].T)
    return blk


def _fill_x_block(blk, x):
    x = np.asarray(x, F32).reshape(-1)
    blk[:, _C_V0:] = 0.0
    blk[0, _C_V0] = 1.0                 # constant-1 slot (v0[0] structural 0)
    blk[2:42, _C_V0] = x[7:47]
    blk[44:84, _C_V0] = x[53:93]
    blk[0, _C_EXT] = x[47]
    blk[1, _C_EXT] = x[52]
    blk[2, _C_EXT] = 1.0                # bias slot for layer 5
    blk[0, _C_DEN] = x[3]               # qw
    blk[1, _C_DEN] = x[5]               # qy
    blk[2, _C_DEN] = x[3]
    blk[3, _C_DEN] = x[5]
    blk[0, _C_NUM] = x[6]               # qz
    blk[1, _C_NUM] = x[4]               # qx
    blk[2, _C_NUM] = x[6]
    blk[3, _C_NUM] = x[4]
    return blk


_CACHE = {}


def _build_bass(debug=False):
    if ("nc", debug) in _CACHE:
        return _CACHE[("nc", debug)]

    import concourse.mybir as mybir
    from concourse import bacc

    f32 = mybir.dt.float32
    bf16 = mybir.dt.bfloat16
    AF = mybir.ActivationFunctionType
    OP = mybir.AluOpType

    class _OneSetBacc(bacc.Bacc):
        """Force every activation to resolve to sigmoid_and_others (it covers
        Tanh/Arctan) so only one ACT table load is paid."""

        def insert_act_table_loads(self):
            import bass_rust as _bass_rust
            from concourse.hw_specs import get_activation_tables

            has_activation = any(
                isinstance(i, mybir.InstActivation)
                for b in self.main_func.blocks
                for i in b.instructions
            )
            if not has_activation:
                return
            tables = list(get_activation_tables(self.m.arch).items())
            ours = dict(tables)["sigmoid_and_others"]
            for f in (AF.Tanh, AF.Arctan):
                assert f in ours, f
            tables = [(n, (fns if n == "sigmoid_and_others" else fns - ours))
                      for n, fns in tables]
            _bass_rust.insert_act_table_loads(self, tables)
            # The pass inserts the load at the head of the ACT stream, ahead
            # of the input-DMA trigger; swap them so the DMA flight hides the
            # table load instead of the load delaying the DMA.
            blk0 = self.main_func.blocks[0]
            insts = blk0.instructions
            li = next(i for i, x in enumerate(insts)
                      if isinstance(x, mybir.InstLoadActFuncSet))
            di = next(i for i, x in enumerate(insts)
                      if isinstance(x, mybir.InstDMACopy)
                      and x.engine == mybir.EngineType.Activation)
            if li < di:
                load = insts[li]
                insts.remove(load)
                di = insts.index(next(
                    x for x in insts if isinstance(x, mybir.InstDMACopy)
                    and x.engine == mybir.EngineType.Activation))
                insts.insert(di + 1, load)

    nc = _OneSetBacc("TRN2", num_devices=8)

    # Drop the init const-AP memsets + all-engine barrier: nothing below
    # references the const tensors (activation biases are APs into _C_Z),
    # so engines can enter kernel work straight from the runtime preamble.
    blk0 = nc.main_func.blocks[0]
    for ins in list(blk0.instructions):
        if not isinstance(ins, mybir.InstCall):
            blk0.instructions.remove(ins)

    b_dram = nc.declare_dram_parameter("blk", [_B_ROWS, _B_COLS], bf16,
                                       isOutput=False)
    out_dram = nc.declare_dram_parameter("out", [40, 1], f32, isOutput=True)

    Bt = nc.alloc_sbuf_tensor("Bt", [128, _B_COLS], bf16)
    Ht = nc.alloc_sbuf_tensor("Ht", [128, 8], bf16)
    St = nc.alloc_sbuf_tensor("St", [128, 8], bf16)
    Ot = nc.alloc_sbuf_tensor("Ot", [1, 40], f32)
    Zt = nc.alloc_sbuf_tensor("Zt", [40, 1], f32)   # zero bias column

    dims = [28, 28, 12, 3, 12, 12, 28]
    ps = [nc.alloc_psum_tensor(f"ps{i}", [m, 1], f32)
          for i, m in enumerate(dims)]
    # The last layer runs with swapped operand roles (stationary = the h7/v0
    # column, moving = the KxM weight block) so the result lands as a [1,40]
    # row: the output DMA then moves one contiguous 160B descriptor instead
    # of 40 scattered 4B partition writes.
    ps.append(nc.alloc_psum_tensor("ps7", [1, 40], f32))

    sem = {n: nc.alloc_semaphore(n) for n in
           ["s_in", "p1", "p2", "p3", "p4", "p5", "p6", "p7", "p8",
            "h1", "h2", "h3", "h4", "h5", "h6", "h7",
            "dv", "dveq", "psi", "cp", "s_out"]}

    # --- Pool: zero Ht rows 0:32, plant the 1.0 bias row at row 32 (rows
    # 33:128 are never read), zero the f32 activation-bias column; the
    # regions are disjoint so the Q7 cores may process them concurrently --
    nc.gpsimd.memset(Ht[0:_BR, 0:8], 0.0).then_inc(sem["s_in"], 1)
    nc.gpsimd.memset(Ht[_BR:_BR + 1, 0:7], 1.0).then_inc(sem["s_in"], 1)
    nc.gpsimd.memset(Zt[0:40, 0:1], 0.0).then_inc(sem["s_in"], 1)

    # --- Activation: input DMA first (earliest HWDGE-capable preamble), the
    # single ACT table load is auto-inserted right before tanh1 and runs
    # back-to-back with the trigger, both hidden under the DMA flight. -----
    nc.scalar.dma_start(Bt[0:_B_ROWS, :], b_dram[:, :]).then_inc(sem["s_in"], 16)

    zb = lambda m: Zt[0:m, 0:1]

    def tanh(i, m, hcol):
        (nc.scalar.activation(Ht[0:m, hcol:hcol + 1], ps[i][0:m, 0:1],
                              AF.Tanh, bias=zb(m), scale=1.0)
         ._wait_ge(sem[f"p{i + 1}"], 1)
         .then_inc(sem[f"h{hcol + 1}"], 1))

    def mm(i, col, k, m, rhs, start, stop, wait=None, wv=1, inc=False):
        ins = nc.tensor.matmul(ps[i][0:m, 0:1], Bt[0:k, col:col + m],
                               rhs, start=start, stop=stop)
        if wait is not None:
            ins._wait_ge(sem[wait], wv)
        if inc:
            ins.then_inc(sem[f"p{i + 1}"], 1)

    hc = lambda c, k: Ht[0:k, c:c + 1]

    # --- DVE: atan2 prep (7 ops, all partition-base 0), then the final
    # PSUM->SBUF copy. St col4 ends as [q0, q1, c0, c1] with
    # c = m*(2g-1) = 2*m*g - m, m = [den<0], g = [num>=0], built from
    # tensor_tensor ops only (the dual-immediate tensor_scalar mult+add
    # form produces garbage on multi-partition [n,1] layouts). Every
    # dependent edge is sem-chained through `dv`: back-to-back same-engine
    # RAW/WAW through SBUF is not reliably interlocked on DVE. -----------
    _lp = nc.allow_low_precision(reason="bf16 atan2 prep; rel tol 2e-2")
    _lp.__enter__()
    nc.vector.reciprocal(St[0:2, 0:1], Bt[0:2, _C_DEN:_C_DEN + 1]) \
        ._wait_ge(sem["s_in"], 19).then_inc(sem["dv"], 1)
    nc.vector.tensor_scalar(St[0:4, 1:2], Bt[0:4, _C_DEN:_C_DEN + 1],
                            0.0, None, OP.is_lt).then_inc(sem["dv"], 1)  # m
    nc.vector.tensor_scalar(St[0:4, 2:3], Bt[0:4, _C_NUM:_C_NUM + 1],
                            0.0, None, OP.is_ge).then_inc(sem["dv"], 1)  # g
    nc.vector.tensor_mul(St[0:4, 3:4], St[0:4, 1:2], St[0:4, 2:3]) \
        ._wait_ge(sem["dv"], 3).then_inc(sem["dv"], 1)                   # m*g
    nc.vector.tensor_add(St[0:4, 5:6], St[0:4, 3:4], St[0:4, 3:4]) \
        ._wait_ge(sem["dv"], 4).then_inc(sem["dv"], 1)                   # 2mg
    nc.vector.tensor_sub(St[0:4, 4:5], St[0:4, 5:6], St[0:4, 1:2]) \
        ._wait_ge(sem["dv"], 5).then_inc(sem["dv"], 1)                   # c_f
    nc.vector.tensor_mul(St[0:2, 4:5], Bt[0:2, _C_NUM:_C_NUM + 1],
                         St[0:2, 0:1]) \
        ._wait_ge(sem["dv"], 6).then_inc(sem["dveq"], 1)                 # q
    _lp.__exit__(None, None, None)


    # --- PE / ACT chain ----------------------------------------------------
    mm(0, _C_L1, 84, 28, Bt[0:84, _C_V0:_C_V0 + 1], True, True,
       wait="s_in", wv=19, inc=True)
    tanh(0, 28, 0)

    # arctan([q0,q1,c0,c1]) -> psi partitions 0:4 (between tanh2 and tanh3)
    mm(1, _C_L2, 33, 28, hc(0, 33), True, True, wait="h1", inc=True)
    tanh(1, 28, 1)
    (nc.scalar.activation(Ht[0:4, _PSIC:_PSIC + 1], St[0:4, 4:5],
                          AF.Arctan, bias=zb(4), scale=1.0)
     ._wait_ge(sem["dveq"], 1).then_inc(sem["psi"], 1))

    mm(2, _C_L3, 33, 12, hc(1, 33), True, True, wait="h2", inc=True)
    tanh(2, 12, 2)

    mm(4, _C_L5E, 3, 12, Bt[0:3, _C_EXT:_C_EXT + 1], True, False)
    mm(3, _C_L4, 33, 3, hc(2, 33), True, True, wait="h3", inc=True)
    tanh(3, 3, 3)

    mm(4, _C_L5P, 4, 12, hc(_PSIC, 4), False, False, wait="psi")
    mm(4, _C_L5A, 3, 12, hc(3, 3), False, True, wait="h4", inc=True)
    tanh(4, 12, 4)

    mm(5, _C_L6H3, 12, 12, hc(2, 12), True, False)
    mm(5, _C_L6H5, 33, 12, hc(4, 33), False, True, wait="h5", inc=True)
    tanh(5, 12, 5)

    mm(6, _C_L7H2, 33, 28, hc(1, 33), True, False)
    mm(6, _C_L7H6, 12, 28, hc(5, 12), False, True, wait="h6", inc=True)
    tanh(6, 28, 6)

    nc.tensor.matmul(ps[7][0:1, 0:40], Bt[0:84, _C_V0:_C_V0 + 1],
                     Bt[0:84, _C_L8B:_C_L8B + 40], start=True, stop=False)
    (nc.tensor.matmul(ps[7][0:1, 0:40], hc(6, 28),
                      Bt[0:28, _C_L8A:_C_L8A + 40], start=False, stop=True)
     ._wait_ge(sem["h7"], 1).then_inc(sem["p8"], 1))

    # final copy on DVE (keeps the ACT queue short), then output DMA on SP
    nc.vector.tensor_scalar(Ot[0:1, 0:40], ps[7][0:1, 0:40],
                            1.0, None, OP.mult) \
        ._wait_ge(sem["p8"], 1).then_inc(sem["cp"], 1)
    # Output DMA from the Activation engine: its HWDGE queue observed a
    # much faster trigger->completion latency than SP's in traces, and the
    # ACT stream is idle after tanh7. Skipping the completion wait outright
    # is not an option (NRT_EXEC_UNIT_UNRECOVERABLE if the NEFF exits with
    # an unwaited in-flight DMA).
    nc.scalar.dma_start(out_dram[:, :], Ot[0:1, 0:40]) \
        ._wait_ge(sem["cp"], 1).then_inc(sem["s_out"], 16)

    s_out_target = 16
    if debug:
        dbg_bt = nc.declare_dram_parameter("dbg_bt", [128, _B_COLS], f32,
                                           isOutput=True)
        dbg_ht = nc.declare_dram_parameter("dbg_ht", [128, 8], f32,
                                           isOutput=True)
        dbg_st = nc.declare_dram_parameter("dbg_st", [128, 8], f32,
                                           isOutput=True)
        nc.scalar.dma_start(dbg_bt[:, :], Bt[:, :]).then_inc(sem["s_out"], 16)
        nc.scalar.dma_start(dbg_ht[:, :], Ht[:, :]).then_inc(sem["s_out"], 16)
        nc.scalar.dma_start(dbg_st[:, :], St[:, :]).then_inc(sem["s_out"], 16)
        s_out_target = 64

    # --- tail: clear kernel sems once the output DMA has completed --------
    from concourse.bass import compact_to_ranges
    nc.gpsimd.wait_ge(sem["s_out"], s_out_target)
    for r in compact_to_ranges(sorted(s.num for s in sem.values())):
        nc.gpsimd.sem_clear(r)

    nc.compile()
    _CACHE[("nc", debug)] = nc
    return nc


def _build_blk(inputs):
    import ml_dtypes
    blk = _build_w_block(inputs)
    _fill_x_block(blk, inputs["x"])
    return blk[:_B_ROWS].astype(ml_dtypes.bfloat16)


def kernel(**inputs) -> np.ndarray:
    nc = _build_bass()
    blk = _build_blk(inputs)

    from concourse.bass_utils import run_bass_kernel_spmd

    res = run_bass_kernel_spmd(nc, [{"blk": blk.copy()} for _ in range(8)],
                               core_ids=list(range(8)))
    out = np.asarray(res.results[0]["out"], F32).reshape(1, 40)
    return out
